# revision 1
# baseline (speedup 1.0000x reference)
"""CRPS loss kernel for Trainium2 (8 NeuronCores, batch-parallel).

Math (per grid point, N=32 ensemble members x_i, target y):
  term1 = (1/N) sum_i |x_i - y|
  term2 = (1/N^2) sum_i (2i+1-N) x_sorted_i          (reference sorts)
        = (1/N^2) (2 sum_{i<j} max(x_i, x_j) - (N-1) sum_i x_i)
  CRPS  = term1 - term2
Latitude weights w_h > 0 factor out of every term; the host applies them
and the final mean in float64.  sum_i x_i is linear -> host f64.

Three-engine pipeline per core (2 of 16 batches, SBUF [h=121, b=2, n=32,
w=240] bf16): the vector engine computes the 31 shifted pairwise maxes
(bf16 2x) into a 2-slot ring of uniform [b,31,w] slots (shift pairs
(d, 33-d) fill exactly 31 rows); the TENSOR engine accumulates every
slot into one f32 PSUM bank via identity-matmul accumulate (measured
0.236 ns/col - more than twice DVE's rate, so it never gates the ring);
the scalar engine does |x-y| (Abs accum) and the single final PSUM
reduction.  DVE is the sole bottleneck (~112 us of maxes + subtract).

Outputs per core: [121, 2] f32 rows {sum|x-y|, sum pairwise max} per
latitude; host combines with the f64 input sum.
"""

import numpy as np
import ml_dtypes

import concourse.bass as bass
import concourse.mybir as mybir
from concourse.bass_utils import run_bass_kernel_spmd

H, W, B, N = 121, 240, 16, 32
N_CORES = 8
B_LOC = B // N_CORES

F32 = mybir.dt.float32
BF16 = mybir.dt.bfloat16
FP8 = mybir.dt.float8e4
ALU = mybir.AluOpType
AFT = mybir.ActivationFunctionType

# ring items: d=1 alone fills a [b,31,w] slot; pairs (d, 33-d) fill the rest
ITEMS = [(1,)] + [(d, 33 - d) for d in range(2, 17)]  # 16 items
CH = 512                                              # psum chunk columns

_NC_CACHE = {}


def build_nc(repeat=1, detect_races=True):
    key = (repeat, detect_races)
    if key in _NC_CACHE:
        return _NC_CACHE[key]
    nc = bass.Bass(detect_race_conditions=detect_races)
    x_in = nc.declare_dram_parameter("x", [H, B_LOC * N * W], BF16, isOutput=False)
    y_in = nc.declare_dram_parameter("y", [H, B_LOC * W], BF16, isOutput=False)
    i_in = nc.declare_dram_parameter("ident", [H, H], BF16, isOutput=False)
    o_out = nc.declare_dram_parameter("o", [H, 2], F32, isOutput=True)

    NI = len(ITEMS)                  # 16
    V = NI + 1                       # v_sem incs per iteration
    FLAT = B_LOC * (N - 1) * W       # 14880 slot columns
    NCH = (FLAT + CH - 1) // CH      # 30 chunks per slot

    with (
        nc.sbuf_tensor([H, B_LOC, N, W], BF16) as xt,
        nc.sbuf_tensor([H, B_LOC, W], BF16) as yt,
        nc.sbuf_tensor([H, H], BF16) as ident,
        nc.sbuf_tensor([H, B_LOC, N - 1, W], BF16) as mxa,
        nc.sbuf_tensor([H, B_LOC, N - 1, W], BF16) as mxb,
        nc.sbuf_tensor([H, B_LOC, N, W], BF16) as dif,
        nc.sbuf_tensor([H, B_LOC, N, W], FP8) as dump_dif,
        nc.sbuf_tensor([H, CH], F32) as dump,
        nc.sbuf_tensor([H, 1], F32) as a1,
        nc.sbuf_tensor([H, 1], F32) as ot_a2,
        nc.sbuf_tensor([H, 2], F32) as ot,
        nc.psum_tensor([H, CH], F32) as p0,
        nc.semaphore() as dma_sem,
        nc.semaphore() as v_sem,
        nc.semaphore() as p_sem,
        nc.semaphore() as s_sem,
        nc.Block() as block,
    ):
        xv = xt[:]
        ring = [mxa[:], mxb[:]]
        ring_flat = [
            mxa[:].rearrange("h b n w -> h (b n w)"),
            mxb[:].rearrange("h b n w -> h (b n w)"),
        ]

        @block.sync
        def _(sync):
            sync.dma_start(
                out=xt[:],
                in_=x_in[:].rearrange("h (b n w) -> h b n w", b=B_LOC, n=N, w=W),
            ).then_inc(dma_sem, 16)
            sync.dma_start(
                out=yt[:],
                in_=y_in[:].rearrange("h (b w) -> h b w", b=B_LOC, w=W),
            ).then_inc(dma_sem, 16)
            sync.dma_start(out=ident[:], in_=i_in[:]).then_inc(dma_sem, 16)
            sync.wait_ge(s_sem, repeat)
            sync.dma_start(out=o_out[:], in_=ot[:]).then_inc(dma_sem, 16)

        @block.vector
        def _(vector):
            vector.wait_ge(dma_sem, 48)
            ybc = yt[:].unsqueeze(2).broadcast_to((H, B_LOC, N, W))
            for it in range(repeat):
                vb = V * it
                pb = NI * it
                if it > 0:
                    vector.wait_ge(s_sem, it)  # prev ACT abs + psum-read done
                nc.vector.tensor_tensor(
                    dif[:], xv, ybc, op=ALU.subtract
                ).then_inc(v_sem, 1)  # vb+1
                for k, item in enumerate(ITEMS):
                    if k >= 2:
                        vector.wait_ge(p_sem, pb + k - 1)  # PE freed slot k-2
                    slot = ring[k % 2]
                    if len(item) == 1:
                        d = item[0]
                        nc.vector.tensor_max(
                            slot[:, :, : N - d, :],
                            xv[:, :, d:, :],
                            xv[:, :, : N - d, :],
                        ).then_inc(v_sem, 1)  # vb+2+k
                    else:
                        da, db = item
                        nc.vector.tensor_max(
                            slot[:, :, : N - da, :],
                            xv[:, :, da:, :],
                            xv[:, :, : N - da, :],
                        )
                        nc.vector.tensor_max(
                            slot[:, :, N - da : N - 1, :],
                            xv[:, :, db:, :],
                            xv[:, :, : N - db, :],
                        ).then_inc(v_sem, 1)  # vb+2+k

        @block.tensor
        def _(tensor):
            tensor.wait_ge(dma_sem, 48)
            for it in range(repeat):
                vb = V * it
                if it > 0:
                    tensor.wait_ge(s_sem, it)  # ACT read psum of prev iter
                for k in range(NI):
                    tensor.wait_ge(v_sem, vb + 2 + k)
                    sf = ring_flat[k % 2]
                    for c in range(NCH):
                        lo = c * CH
                        hi = min(FLAT, lo + CH)
                        mm = tensor.matmul(
                            p0[:, : hi - lo],
                            ident[:],
                            sf[:, lo:hi],
                            start=(k == 0 and c == 0),
                            stop=(k == NI - 1 and c == NCH - 1),
                        )
                    mm.then_inc(p_sem, 1)  # slot k consumed

        @block.scalar
        def _(scalar):
            for it in range(repeat):
                vb = V * it
                scalar.wait_ge(v_sem, vb + 1)
                nc.scalar.activation(dump_dif[:], dif[:], AFT.Abs, accum_out=a1[:])
                scalar.wait_ge(p_sem, NI * (it + 1))  # all slots accumulated
                nc.scalar.activation(dump[:], p0[:], AFT.Copy, accum_out=ot_a2[:])
                nc.scalar.copy(ot[:, 0:1], a1[:])
                nc.scalar.copy(ot[:, 1:2], ot_a2[:]).then_inc(s_sem, 1)

    _NC_CACHE[key] = nc
    return nc


def _prep_inputs(predictions, targets):
    """Full f32 [B,N,H,W]/[B,H,W] -> per-core bf16 maps, layout [h,b,n,w]."""
    p = np.asarray(predictions, dtype=np.float32)
    t = np.asarray(targets, dtype=np.float32)
    pt = np.ascontiguousarray(p.transpose(2, 0, 1, 3)).astype(ml_dtypes.bfloat16)
    tt = np.ascontiguousarray(t.transpose(1, 0, 2)).astype(ml_dtypes.bfloat16)
    ident = np.eye(H).astype(ml_dtypes.bfloat16)
    in_maps = []
    for c in range(N_CORES):
        xc = np.ascontiguousarray(pt[:, B_LOC * c : B_LOC * (c + 1)]).reshape(
            H, B_LOC * N * W
        )
        yc = np.ascontiguousarray(tt[:, B_LOC * c : B_LOC * (c + 1)]).reshape(
            H, B_LOC * W
        )
        in_maps.append({"x": xc, "y": yc, "ident": ident})
    return in_maps


def _lat_weights_f64():
    lats = np.arange(90.0, -91.5, -1.5)  # [121]
    w = np.cos(np.deg2rad(lats))
    return H * (w / np.sum(w))


def _combine(outs, predictions):
    """outs: list of [H,2] f32 -> scalar f32 (host math in f64)."""
    w = _lat_weights_f64()
    p = np.asarray(predictions, dtype=np.float32)
    pb = p.astype(ml_dtypes.bfloat16).astype(np.float64)  # match device rounding
    a3_h = pb.sum(axis=(0, 1, 3))  # [H]
    a1_h = np.zeros(H, np.float64)
    a2_h = np.zeros(H, np.float64)
    for o in outs:
        o = np.asarray(o, dtype=np.float64)
        a1_h += o[:, 0]
        a2_h += o[:, 1]
    s2 = 2.0 * a2_h - (N - 1) * a3_h
    crps_h = a1_h / N - s2 / (N * N)
    total = float(np.dot(w, crps_h))
    return np.float32(total / (B * H * W))


def kernel(predictions, targets):
    nc = build_nc()
    in_maps = _prep_inputs(predictions, targets)
    res = run_bass_kernel_spmd(nc, in_maps, list(range(N_CORES)))
    outs = [res.results[i]["o"] for i in range(N_CORES)]
    return _combine(outs, predictions)

